# revision 2
# baseline (speedup 1.0000x reference)
"""Self-contained TRN2 Bass kernel for nn_GCL (2-layer GCN + projection),
running SPMD across 8 NeuronCores.

  h1 = relu(Ahat @ (x @ W1) + b1)
  h2 = Ahat @ (h1 @ W2) + b2
  out = h2 @ Wp + bp,   Ahat = D^-1/2 (A+I) D^-1/2, deg = indeg(dst)+1

Strategy (graph/data parallel, dst-sharded):
  * Nodes are sharded contiguously across 8 cores; edges are bucketed by
    (dst core, dst block of 112, src-table half), sorted by src, padded to
    T*128 entries per bucket so the SPMD program is identical on all cores.
  * Per layer: each core computes z = prev @ W for its nodes (TensorE),
    scales rows by dis = deg^-1/2, and AllGathers the scaled table (bf16).
  * Aggregation: dma_gather pulls 512B rows from the table (4 SWDGE queues);
    a one-hot S matrix built on-chip (is_equal vs an iota tile) turns the
    per-edge rows into per-dst segment sums on the TensorE, accumulating in
    PSUM together with an identity-matmul self-loop term. The dst-side
    dis scale + bias + activation run on DVE/ScalarE out of PSUM.
  * int16 gather indices address the table as two halves (offset views).

Compute dtype bf16 (fp32 PSUM accumulation); final output fp32.
"""

from contextlib import ExitStack

import numpy as np
import ml_dtypes

NPBF16 = ml_dtypes.bfloat16
NPFP8 = ml_dtypes.float8_e3m4

# problem geometry (from the problem spec)
N_NODES, N_EDGES = 50000, 800000
IN_DIM, HID_DIM, OUT_DIM = 512, 256, 256
N_CORES = 8


class _P:
    def __init__(self, T, block_dst=112, gather_queues=4, gather_bufs=12,
                 scratch=16384, sbufs=4, hbufs=3, sp=None, rtcnt=0,
                 g0=False, c0=False, s0=False, m0=False, vtag=0, reps=1,
                 mb=1, ck=False, fp8=False):
        self.g0, self.c0, self.s0, self.m0 = g0, c0, s0, m0
        self.vtag = vtag
        self.reps = reps
        self.mb = mb
        self.ck = ck
        self.fp8 = fp8
        self.n_nodes = N_NODES
        self.in_dim = IN_DIM
        self.F = HID_DIM
        self.n_cores = N_CORES
        self.npc = N_NODES // N_CORES
        self.bd = block_dst
        self.blocks = -(-self.npc // block_dst)
        self.slots = self.blocks * block_dst
        self.tbl_rows = N_CORES * self.slots
        self.half = (self.tbl_rows // 2 + 255) // 256 * 256
        assert self.half < 32768 and self.tbl_rows - self.half < 32768
        self.T = T
        self.NI = T * 128
        self.ncalls = self.blocks * 2
        self.kin = IN_DIM // 128
        self.kf = self.F // 128
        self.gq = gather_queues
        self.gbufs = gather_bufs
        self.scratch = scratch
        self.sbufs = sbufs
        self.hbufs = hbufs
        self.single_packet = (self.NI * mb <= 1024) if sp is None else bool(sp)
        self.rtcnt = rtcnt


def _src_map(p, src):
    core = src // p.npc
    slot = src % p.npc
    if not p.ck:
        return core * p.slots + slot
    ch = p.slots // 2
    return np.where(slot < ch, core * ch + slot,
                    p.half + core * ch + (slot - ch))


def _build_kernel(p):
    import concourse.bacc as bacc
    import concourse.mybir as mybir
    import concourse.tile as tile

    BF16, F32, I16 = mybir.dt.bfloat16, mybir.dt.float32, mybir.dt.int16
    TDT = mybir.dt.float8e3 if p.fp8 else BF16

    nc = bacc.Bacc("TRN2", target_bir_lowering=False, debug=False,
                   num_devices=p.n_cores, num_swdge_queues=p.gq,
                   dynamic_dma_scratch_size=p.scratch)

    xT = nc.dram_tensor("xT", [p.in_dim, p.slots], BF16, kind="ExternalInput")
    W1 = nc.dram_tensor("W1", [p.in_dim, p.F], BF16, kind="ExternalInput")
    W2 = nc.dram_tensor("W2", [p.F, p.F], BF16, kind="ExternalInput")
    Wp = nc.dram_tensor("Wp", [p.F, p.F], BF16, kind="ExternalInput")
    b1b = nc.dram_tensor("b1b", [128, p.F], F32, kind="ExternalInput")
    bpeb = nc.dram_tensor("bpeb", [128, p.F], F32, kind="ExternalInput")
    discol = nc.dram_tensor("discol", [128, p.blocks], F32, kind="ExternalInput")
    ddiag = nc.dram_tensor("ddiag", [p.blocks * 128, p.bd], TDT, kind="ExternalInput")
    disinv = nc.dram_tensor("disinv", [1, p.blocks * p.bd], BF16, kind="ExternalInput")
    b1r = nc.dram_tensor("b1r", [1, p.F], BF16, kind="ExternalInput")
    iota = nc.dram_tensor("iota", [128, 128], BF16, kind="ExternalInput")
    ident = nc.dram_tensor("ident", [128, 128], BF16, kind="ExternalInput")
    vtag = None
    if p.vtag:
        vtag = nc.dram_tensor("vtag", [1, p.vtag], F32, kind="ExternalInput")
    idx = nc.dram_tensor("idx", [128, p.ncalls * (p.NI // 16)], I16, kind="ExternalInput")
    seg = nc.dram_tensor("seg", [128, p.ncalls * p.T], F32, kind="ExternalInput")
    out = nc.dram_tensor("out", [p.slots, p.F], F32, kind="ExternalOutput")

    with tile.TileContext(nc) as tc, ExitStack() as ctx:
        const = ctx.enter_context(tc.tile_pool(name="const", bufs=1))
        dram = ctx.enter_context(tc.tile_pool(name="dram", bufs=1, space="DRAM"))
        zpool = ctx.enter_context(tc.tile_pool(name="z", bufs=1))
        gpool = ctx.enter_context(tc.tile_pool(name="g", bufs=p.gbufs))
        spool = ctx.enter_context(tc.tile_pool(name="s", bufs=p.sbufs))
        hpool = ctx.enter_context(tc.tile_pool(name="h", bufs=p.hbufs))
        xpool = ctx.enter_context(tc.tile_pool(name="x", bufs=3))
        ppool = ctx.enter_context(tc.tile_pool(name="ps", bufs=max(3, p.mb + 1),
                                               space="PSUM"))
        p2pool = ctx.enter_context(tc.tile_pool(name="ps2", bufs=2, space="PSUM"))
        tppool = ctx.enter_context(tc.tile_pool(name="pst", bufs=1, space="PSUM"))

        w1_t = const.tile([128, p.kin * p.F], BF16)
        for k in range(p.kin):
            nc.sync.dma_start(w1_t[:, k * p.F:(k + 1) * p.F], W1[k * 128:(k + 1) * 128, :])
        w2_t = const.tile([128, p.kf * p.F], BF16)
        for k in range(p.kf):
            nc.sync.dma_start(w2_t[:, k * p.F:(k + 1) * p.F], W2[k * 128:(k + 1) * 128, :])
        wp_t = const.tile([128, p.kf * p.F], BF16)
        for k in range(p.kf):
            nc.sync.dma_start(wp_t[:, k * p.F:(k + 1) * p.F], Wp[k * 128:(k + 1) * 128, :])
        b1_t = const.tile([128, p.F], F32)
        nc.sync.dma_start(b1_t[:], b1b[:])
        bpe_t = const.tile([128, p.F], F32)
        nc.sync.dma_start(bpe_t[:], bpeb[:])
        dis_t = const.tile([128, p.blocks], F32)
        nc.sync.dma_start(dis_t[:], discol[:])
        ddiag_t = const.tile([128, p.blocks * p.bd], TDT)
        for b in range(p.blocks):
            nc.sync.dma_start(ddiag_t[:, b * p.bd:(b + 1) * p.bd],
                              ddiag[b * 128:(b + 1) * 128, :])
        disinv_t = const.tile([1, p.blocks * p.bd], BF16)
        nc.sync.dma_start(disinv_t[:], disinv[:])
        b1r_t = const.tile([1, p.F], BF16)
        nc.sync.dma_start(b1r_t[:], b1r[:])
        if vtag is not None:
            vtag_t = const.tile([1, p.vtag], F32)
            nc.sync.dma_start(vtag_t[:], vtag[:])
        iota_t = const.tile([128, 128], BF16)
        nc.sync.dma_start(iota_t[:], iota[:])
        ident_t = const.tile([128, 128], BF16)
        nc.sync.dma_start(ident_t[:], ident[:])
        idx_t = const.tile([128, p.ncalls * (p.NI // 16)], I16)
        nc.sync.dma_start(idx_t[:], idx[:])
        seg_t = const.tile([128, p.ncalls * p.T], F32)
        nc.sync.dma_start(seg_t[:], seg[:])

        z1loc = zpool.tile([128, p.blocks * p.F], TDT, tag="z1")
        z2loc = zpool.tile([128, p.blocks * p.F], TDT, tag="z2")

        zbounce1 = dram.tile([p.slots, p.F], TDT, tag="zb1")
        zbounce2 = dram.tile([p.slots, p.F], TDT, tag="zb2")
        ch = p.slots // 2
        bh = p.blocks // 2
        if p.ck:
            tab1a = nc.dram_tensor("tab1a", [p.half, p.F], TDT,
                                   kind="Internal", addr_space="Shared").ap()
            tab1b = nc.dram_tensor("tab1b", [p.half, p.F], TDT,
                                   kind="Internal", addr_space="Shared").ap()
            tab2a = nc.dram_tensor("tab2a", [p.half, p.F], TDT,
                                   kind="Internal", addr_space="Shared").ap()
            tab2b = nc.dram_tensor("tab2b", [p.half, p.F], TDT,
                                   kind="Internal", addr_space="Shared").ap()
            acc = zpool.tile([128, p.blocks * p.F], BF16, tag="acc")
        else:
            tab1 = nc.dram_tensor("tab1", [p.tbl_rows, p.F], TDT,
                                  kind="Internal", addr_space="Shared").ap()
            tab2 = nc.dram_tensor("tab2", [p.tbl_rows, p.F], TDT,
                                  kind="Internal", addr_space="Shared").ap()

        def allgather(src_ap, dst_ap):
            if p.c0:
                return
            nc.gpsimd.collective_compute(
                "AllGather", mybir.AluOpType.bypass,
                replica_groups=[list(range(p.n_cores))],
                ins=[src_ap], outs=[dst_ap])

        def produce_z1():
            for b in range(p.blocks):
                xt = xpool.tile([128, p.kin * p.bd], BF16, tag="xt")
                for k in range(p.kin):
                    nc.sync.dma_start(
                        xt[:, k * p.bd:(k + 1) * p.bd],
                        xT[k * 128:(k + 1) * 128, b * p.bd:(b + 1) * p.bd])
                ps = p2pool.tile([p.bd, p.F], F32, tag="zps")
                for k in range(p.kin):
                    nc.tensor.matmul(ps[:], xt[:, k * p.bd:(k + 1) * p.bd],
                                     w1_t[:, k * p.F:(k + 1) * p.F],
                                     start=(k == 0), stop=(k == p.kin - 1))
                zb = z1loc[:p.bd, b * p.F:(b + 1) * p.F]
                nc.vector.tensor_scalar_mul(zb, ps[:], dis_t[:p.bd, b:b + 1])
                nc.sync.dma_start(zbounce1[b * p.bd:(b + 1) * p.bd, :], zb)
                if p.ck and b == bh - 1:
                    allgather(zbounce1[0:ch, :].opt(), tab1a)
            if p.ck:
                allgather(zbounce1[ch:, :].opt(), tab1b)

        iota_b = iota_t[:].rearrange("p f -> p () f").broadcast_to([128, p.T, 128])

        gfix = sfix = None
        if p.g0:
            gfix = const.tile([128, p.mb * p.T, p.F], TDT)
            nc.vector.memset(gfix[:], 0.0)
        if p.s0:
            sfix = const.tile([128, p.T, 128], TDT)
            nc.vector.memset(sfix[:], 0.0)

        def agg_super(layer_tab, zloc, g, qc, l1=False):
            MB = p.mb
            gts = []
            for half in (0, 1):
                call = g * 2 + half
                if p.g0:
                    gts.append(gfix)
                    continue
                gt = gpool.tile([128, MB * p.T, p.F], TDT, tag="g")
                src = layer_tab[:p.half, :] if half == 0 else layer_tab[p.half:, :]
                w = MB * p.NI // 16
                nc.gpsimd.dma_gather(
                    gt[:], src, idx_t[:, call * w:(call + 1) * w],
                    MB * p.NI, MB * p.NI, p.F, queue_num=qc[0] % p.gq,
                    single_packet=p.single_packet)
                qc[0] += 1
                gts.append(gt)
            out_ps = []
            for bi in range(MB):
                b = g * MB + bi
                ps = ppool.tile([p.bd, p.F], F32, tag="agg")
                mm = 0
                for half, gt in ((0, gts[0]), (1, gts[1])):
                    sidx = (g * 2 + half) * MB + bi
                    if p.s0:
                        s = sfix
                    else:
                        s = spool.tile([128, p.T, 128], TDT, tag="s")
                        seg_b = seg_t[:, sidx * p.T:(sidx + 1) * p.T] \
                            .rearrange("p t -> p t ()").broadcast_to([128, p.T, 128])
                        nc.vector.tensor_tensor(s[:], iota_b, seg_b,
                                                op=mybir.AluOpType.is_equal)
                    if not p.m0:
                        for t in range(p.T):
                            nc.tensor.matmul(ps[:], s[:, t, :p.bd],
                                             gt[:, bi * p.T + t, :],
                                             start=(mm == 0), stop=False)
                            mm += 1
                if p.m0:
                    nc.tensor.matmul(ps[:], sfix[:, 0, :p.bd] if p.s0 else
                                     ddiag_t[:p.bd, b * p.bd:(b + 1) * p.bd],
                                     gfix[:, 0, :] if p.g0 else
                                     zloc[:p.bd, b * p.F:(b + 1) * p.F],
                                     start=True, stop=False)
                if l1:
                    nc.tensor.matmul(ps[:], disinv_t[:1, b * p.bd:(b + 1) * p.bd],
                                     b1r_t[:1, :], start=False, stop=False)
                nc.tensor.matmul(ps[:], ddiag_t[:p.bd, b * p.bd:(b + 1) * p.bd],
                                 zloc[:p.bd, b * p.F:(b + 1) * p.F],
                                 start=False, stop=True)
                out_ps.append((b, ps))
            return out_ps

        def gather_call(tab, call, qc):
            MB = p.mb
            gt = gpool.tile([128, MB * p.T, p.F], TDT, tag="g")
            w = MB * p.NI // 16
            nc.gpsimd.dma_gather(
                gt[:], tab, idx_t[:, call * w:(call + 1) * w],
                MB * p.NI, MB * p.NI, p.F, queue_num=qc[0] % p.gq,
                single_packet=p.single_packet)
            qc[0] += 1
            return gt

        def sbuild(sidx):
            s = spool.tile([128, p.T, 128], TDT, tag="s")
            seg_b = seg_t[:, sidx * p.T:(sidx + 1) * p.T] \
                .rearrange("p t -> p t ()").broadcast_to([128, p.T, 128])
            nc.vector.tensor_tensor(s[:], iota_b, seg_b,
                                    op=mybir.AluOpType.is_equal)
            return s

        def agg_phase_a(tabA, g, qc):
            gt = gather_call(tabA, g * 2, qc)
            for bi in range(p.mb):
                b = g * p.mb + bi
                ps = ppool.tile([p.bd, p.F], F32, tag="agg")
                s = sbuild((g * 2) * p.mb + bi)
                for t in range(p.T):
                    nc.tensor.matmul(ps[:], s[:, t, :p.bd],
                                     gt[:, bi * p.T + t, :],
                                     start=(t == 0), stop=(t == p.T - 1))
                nc.scalar.activation(acc[:p.bd, b * p.F:(b + 1) * p.F], ps[:],
                                     mybir.ActivationFunctionType.Copy)

        def agg_phase_b(tabB, zloc, g, qc, l1=False):
            gt = gather_call(tabB, g * 2 + 1, qc)
            out_ps = []
            for bi in range(p.mb):
                b = g * p.mb + bi
                ps = ppool.tile([p.bd, p.F], F32, tag="agg")
                s = sbuild((g * 2 + 1) * p.mb + bi)
                for t in range(p.T):
                    nc.tensor.matmul(ps[:], s[:, t, :p.bd],
                                     gt[:, bi * p.T + t, :],
                                     start=(t == 0), stop=False)
                if l1:
                    nc.tensor.matmul(ps[:], disinv_t[:1, b * p.bd:(b + 1) * p.bd],
                                     b1r_t[:1, :], start=False, stop=False)
                nc.tensor.matmul(ps[:], ddiag_t[:p.bd, b * p.bd:(b + 1) * p.bd],
                                 zloc[:p.bd, b * p.F:(b + 1) * p.F],
                                 start=False, stop=False)
                nc.tensor.matmul(ps[:], ident_t[:p.bd, :p.bd],
                                 acc[:p.bd, b * p.F:(b + 1) * p.F],
                                 start=False, stop=True)
                out_ps.append((b, ps))
            return out_ps

        def transpose_to(src_bf16):
            hT = hpool.tile([128, p.kf * 128], BF16, tag="ht")
            for k in range(p.kf):
                pst = tppool.tile([128, 128], BF16, tag="tps")
                nc.tensor.transpose(pst[:, :p.bd],
                                    src_bf16[:p.bd, k * 128:(k + 1) * 128],
                                    ident_t[:p.bd, :p.bd])
                nc.scalar.activation(hT[:, k * 128:k * 128 + p.bd], pst[:, :p.bd],
                                     mybir.ActivationFunctionType.Copy)
            return hT

        def layer1_block_post(b, ps):
            h1 = hpool.tile([128, p.F], BF16, tag="hb")
            nc.vector.tensor_scalar(h1[:p.bd], ps[:], dis_t[:p.bd, b:b + 1], 0.0,
                                    op0=mybir.AluOpType.mult,
                                    op1=mybir.AluOpType.max)
            h1T = transpose_to(h1)
            ps2 = p2pool.tile([p.bd, p.F], F32, tag="zps")
            for k in range(p.kf):
                nc.tensor.matmul(ps2[:], h1T[:, k * 128:k * 128 + p.bd],
                                 w2_t[:, k * p.F:(k + 1) * p.F],
                                 start=(k == 0), stop=(k == p.kf - 1))
            zb = z2loc[:p.bd, b * p.F:(b + 1) * p.F]
            nc.vector.tensor_scalar_mul(zb, ps2[:], dis_t[:p.bd, b:b + 1])
            nc.sync.dma_start(zbounce2[b * p.bd:(b + 1) * p.bd, :], zb)

        def layer2_block_post(b, ps):
            h2 = hpool.tile([128, p.F], BF16, tag="hb")
            nc.vector.tensor_scalar_mul(h2[:p.bd], ps[:], dis_t[:p.bd, b:b + 1])
            h2T = transpose_to(h2)
            ps2 = p2pool.tile([p.bd, p.F], F32, tag="zps")
            for k in range(p.kf):
                nc.tensor.matmul(ps2[:], h2T[:, k * 128:k * 128 + p.bd],
                                 wp_t[:, k * p.F:(k + 1) * p.F],
                                 start=(k == 0), stop=(k == p.kf - 1))
            of = hpool.tile([p.bd, p.F], F32, tag="of")
            nc.vector.tensor_add(of[:], ps2[:], bpe_t[:p.bd])
            nc.sync.dma_start(out[b * p.bd:(b + 1) * p.bd, :], of[:])

        qc = [0]
        for rep in range(p.reps):
            if p.ck:
                produce_z1()  # emits AG1a mid-loop + AG1b at end
                for g in range(p.blocks // p.mb):
                    agg_phase_a(tab1a, g, qc)
                for g in range(p.blocks // p.mb):
                    for b, ps in agg_phase_b(tab1b, z1loc, g, qc, l1=True):
                        layer1_block_post(b, ps)
                        if b == bh - 1:
                            allgather(zbounce2[0:ch, :].opt(), tab2a)
                allgather(zbounce2[ch:, :].opt(), tab2b)
                for g in range(p.blocks // p.mb):
                    agg_phase_a(tab2a, g, qc)
                for g in range(p.blocks // p.mb):
                    for b, ps in agg_phase_b(tab2b, z2loc, g, qc):
                        layer2_block_post(b, ps)
                continue

            produce_z1()
            if not p.c0:
                nc.gpsimd.collective_compute(
                    "AllGather", mybir.AluOpType.bypass,
                    replica_groups=[list(range(p.n_cores))],
                    ins=[zbounce1.opt()], outs=[tab1])

            for g in range(p.blocks // p.mb):
                for b, ps in agg_super(tab1, z1loc, g, qc, l1=True):
                    layer1_block_post(b, ps)

            if not p.c0:
                nc.gpsimd.collective_compute(
                    "AllGather", mybir.AluOpType.bypass,
                    replica_groups=[list(range(p.n_cores))],
                    ins=[zbounce2.opt()], outs=[tab2])

            for g in range(p.blocks // p.mb):
                for b, ps in agg_super(tab2, z2loc, g, qc):
                    layer2_block_post(b, ps)

    nc.compile()
    return nc


def _wrap_idx(flat, NI):
    a = np.empty((128, NI // 16), np.int16)
    blk = flat.reshape(NI // 16, 16).T
    for g in range(8):
        a[g * 16:(g + 1) * 16, :] = blk
    return a


def _preprocess(p, x, edge_index, W1, b1, W2, b2, Wp, bp):
    n = p.n_nodes
    src = np.asarray(edge_index[0], np.int64)
    dst = np.asarray(edge_index[1], np.int64)

    deg = np.bincount(dst, minlength=n).astype(np.float64) + 1.0
    dis = (1.0 / np.sqrt(deg)).astype(np.float32)

    src_t = _src_map(p, src)
    dst_core = dst // p.npc
    dst_slot = dst % p.npc
    dst_block = dst_slot // p.bd
    dst_seg = dst_slot % p.bd

    halfsel = (src_t >= p.half).astype(np.int64)
    bucket = dst_core * (p.blocks * 2) + dst_block * 2 + halfsel

    order = np.lexsort((src_t, bucket))
    b_sorted = bucket[order]
    s_sorted = src_t[order]
    g_sorted = dst_seg[order]

    nbuckets = p.n_cores * p.blocks * 2
    counts = np.bincount(b_sorted, minlength=nbuckets)
    assert int(counts.max()) <= p.NI
    starts = np.zeros(nbuckets + 1, np.int64)
    np.cumsum(counts, out=starts[1:])

    rtcnt = getattr(p, "rtcnt", 0)
    fill = -1 if rtcnt else 0
    idx_pad = np.full((nbuckets, p.NI), fill, np.int64)
    seg_pad = np.full((nbuckets, p.NI), -1.0, np.float32)
    cnt_arr = np.maximum(counts, 1).astype(np.int32)
    for bkt in range(nbuckets):
        s, e = starts[bkt], starts[bkt + 1]
        loc = s_sorted[s:e] - (p.half if bkt % 2 else 0)
        idx_pad[bkt, :e - s] = loc
        seg_pad[bkt, :e - s] = g_sorted[s:e]
        if rtcnt and e == s:
            idx_pad[bkt, 0] = 0

    W1b = np.asarray(W1, np.float32).astype(NPBF16)
    W2b = np.asarray(W2, np.float32).astype(NPBF16)
    Wpb = np.asarray(Wp, np.float32).astype(NPBF16)
    bpe = (np.asarray(b2, np.float32) @ np.asarray(Wp, np.float32)
           + np.asarray(bp, np.float32))
    b1b = np.broadcast_to(np.asarray(b1, np.float32), (128, p.F)).copy()
    bpeb = np.broadcast_to(bpe, (128, p.F)).copy()
    iota = np.broadcast_to(np.arange(128, dtype=np.float32), (128, 128)).astype(NPBF16)
    ident = np.eye(128, dtype=np.float32).astype(NPBF16)

    x = np.asarray(x, np.float32)
    in_maps = []
    for c in range(p.n_cores):
        lo, hi = c * p.npc, (c + 1) * p.npc
        xl = np.zeros((p.slots, p.in_dim), np.float32)
        xl[:p.npc] = x[lo:hi]
        xT = np.ascontiguousarray(xl.T).astype(NPBF16)

        disl = np.zeros(p.slots, np.float32)
        disl[:p.npc] = dis[lo:hi]
        dv = disl.reshape(p.blocks, p.bd)
        discol = np.zeros((128, p.blocks), np.float32)
        discol[:p.bd, :] = dv.T
        ddiag = np.zeros((p.blocks * 128, p.bd), np.float32)
        for b in range(p.blocks):
            np.fill_diagonal(ddiag[b * 128: b * 128 + p.bd, :],
                             (dv[b] > 0).astype(np.float32))
        ddiag = ddiag.astype(NPFP8 if p.fp8 else NPBF16)

        idx_sb = np.empty((128, p.ncalls * (p.NI // 16)), np.int16)
        seg_sb = np.empty((128, p.ncalls * p.T), np.float32)
        base = c * p.blocks * 2
        w = p.mb * p.NI // 16
        for g in range(p.blocks // p.mb):
            for half in (0, 1):
                gc = g * 2 + half
                flat = np.concatenate(
                    [idx_pad[base + (g * p.mb + bi) * 2 + half]
                     for bi in range(p.mb)])
                idx_sb[:, gc * w:(gc + 1) * w] = \
                    _wrap_idx(flat.astype(np.int16), p.mb * p.NI)
                for bi in range(p.mb):
                    sidx = gc * p.mb + bi
                    seg_sb[:, sidx * p.T:(sidx + 1) * p.T] = \
                        seg_pad[base + (g * p.mb + bi) * 2 + half] \
                        .reshape(p.T, 128).T
        disinv = np.zeros((1, p.slots), np.float32)
        nz = disl > 0
        disinv[0, nz] = 1.0 / disl[nz]
        extra = {}
        if p.vtag:
            extra["vtag"] = np.zeros((1, p.vtag), np.float32)
        in_maps.append({
            **extra,
            "xT": xT, "W1": W1b, "W2": W2b, "Wp": Wpb,
            "b1b": b1b, "bpeb": bpeb, "discol": discol, "ddiag": ddiag,
            "disinv": disinv.astype(NPBF16),
            "b1r": np.asarray(b1, np.float32).reshape(1, p.F).astype(NPBF16),
            "iota": iota, "ident": ident, "idx": idx_sb, "seg": seg_sb,
            "cnt": cnt_arr[base:base + p.ncalls][None, :],
        })
    return in_maps


_CACHE = {}

# chosen config: multi-packet SWDGE gathers (measurably faster than
# single-packet) + a small dummy input that keeps this program's HLO
# distinct from prior builds sharing the same tensor shapes.
FLAGS = dict(sp=False, vtag=24)


def kernel(x, edge_index, W1, b1, W2, b2, Wp, bp):
    """Full inputs in, full output out. Shards across 8 NeuronCores inside."""
    from concourse.bass_utils import run_bass_kernel_spmd

    src = np.asarray(edge_index[0], np.int64)
    dst = np.asarray(edge_index[1], np.int64)

    # choose T (gather tiles per bucket) from the actual edge distribution;
    # identical across cores so the SPMD program stays uniform.
    p0 = _P(T=1)
    src_t = _src_map(p0, src)
    halfsel = (src_t >= p0.half).astype(np.int64)
    dst_slot = dst % p0.npc
    bucket = ((dst // p0.npc) * (p0.blocks * 2)
              + (dst_slot // p0.bd) * 2 + halfsel)
    counts = np.bincount(bucket, minlength=p0.n_cores * p0.blocks * 2)
    T = max(1, -(-int(counts.max()) // 128))
    p = _P(T=T, **FLAGS)

    key = (T,)
    if key not in _CACHE:
        _CACHE[key] = _build_kernel(p)
    nc = _CACHE[key]

    in_maps = _preprocess(p, x, edge_index, W1, b1, W2, b2, Wp, bp)
    res = run_bass_kernel_spmd(nc, in_maps, core_ids=list(range(p.n_cores)))
    parts = [np.asarray(res.results[c]["out"])[:p.npc] for c in range(p.n_cores)]
    return np.concatenate(parts, axis=0).astype(np.float32)



# revision 3
# speedup vs baseline: 1.3841x; 1.3841x over previous
"""Self-contained TRN2 Bass kernel for nn_GCL (2-layer GCN + projection),
running SPMD across 8 NeuronCores.

  h1 = relu(Ahat @ (x @ W1) + b1)
  h2 = Ahat @ (h1 @ W2) + b2
  out = h2 @ Wp + bp,   Ahat = D^-1/2 (A+I) D^-1/2, deg = indeg(dst)+1

Strategy (graph/data parallel, dst-sharded):
  * Nodes are sharded contiguously across 8 cores; edges are bucketed by
    (dst core, dst block of 112, src-table half), sorted by src, padded to
    T*128 entries per bucket so the SPMD program is identical on all cores.
  * Per layer: each core computes z = prev @ W for its nodes (TensorE),
    scales rows by dis = deg^-1/2, and AllGathers the scaled table (bf16).
  * Aggregation: dma_gather pulls 512B rows from the table (4 SWDGE queues);
    a one-hot S matrix built on-chip (is_equal vs an iota tile) turns the
    per-edge rows into per-dst segment sums on the TensorE, accumulating in
    PSUM together with an identity-matmul self-loop term. The dst-side
    dis scale + bias + activation run on DVE/ScalarE out of PSUM.
  * int16 gather indices address the table as two halves (offset views).

Compute dtype bf16 (fp32 PSUM accumulation); final output fp32.
"""

from contextlib import ExitStack

import numpy as np
import ml_dtypes

NPBF16 = ml_dtypes.bfloat16
NPFP8 = ml_dtypes.float8_e3m4

# problem geometry (from the problem spec)
N_NODES, N_EDGES = 50000, 800000
IN_DIM, HID_DIM, OUT_DIM = 512, 256, 256
N_CORES = 8


class _P:
    def __init__(self, T, block_dst=112, gather_queues=4, gather_bufs=12,
                 scratch=16384, sbufs=4, hbufs=3, sp=None, rtcnt=0,
                 g0=False, c0=False, s0=False, m0=False, vtag=0, reps=1,
                 mb=1, ck=False, fp8=False):
        self.g0, self.c0, self.s0, self.m0 = g0, c0, s0, m0
        self.vtag = vtag
        self.reps = reps
        self.mb = mb
        self.ck = ck
        self.fp8 = fp8
        self.n_nodes = N_NODES
        self.in_dim = IN_DIM
        self.F = HID_DIM
        self.n_cores = N_CORES
        self.npc = N_NODES // N_CORES
        self.bd = block_dst
        self.blocks = -(-self.npc // block_dst)
        self.slots = self.blocks * block_dst
        self.tbl_rows = N_CORES * self.slots
        self.half = (self.tbl_rows // 2 + 255) // 256 * 256
        assert self.half < 32768 and self.tbl_rows - self.half < 32768
        self.T = T
        self.NI = T * 128
        self.ncalls = self.blocks * 2
        self.kin = IN_DIM // 128
        self.kf = self.F // 128
        self.gq = gather_queues
        self.gbufs = gather_bufs
        self.scratch = scratch
        self.sbufs = sbufs
        self.hbufs = hbufs
        self.single_packet = (self.NI * mb <= 1024) if sp is None else bool(sp)
        self.rtcnt = rtcnt


def _src_map(p, src):
    core = src // p.npc
    slot = src % p.npc
    if not p.ck:
        return core * p.slots + slot
    ch = p.slots // 2
    return np.where(slot < ch, core * ch + slot,
                    p.half + core * ch + (slot - ch))


def _build_kernel(p):
    import concourse.bacc as bacc
    import concourse.mybir as mybir
    import concourse.tile as tile

    BF16, F32, I16 = mybir.dt.bfloat16, mybir.dt.float32, mybir.dt.int16
    TDT = mybir.dt.float8e3 if p.fp8 else BF16

    nc = bacc.Bacc("TRN2", target_bir_lowering=False, debug=False,
                   num_devices=p.n_cores, num_swdge_queues=p.gq,
                   dynamic_dma_scratch_size=p.scratch)

    xT = nc.dram_tensor("xT", [p.in_dim, p.slots], BF16, kind="ExternalInput")
    W1 = nc.dram_tensor("W1", [p.in_dim, p.F], BF16, kind="ExternalInput")
    W2 = nc.dram_tensor("W2", [p.F, p.F], BF16, kind="ExternalInput")
    Wp = nc.dram_tensor("Wp", [p.F, p.F], BF16, kind="ExternalInput")
    b1b = nc.dram_tensor("b1b", [128, p.F], F32, kind="ExternalInput")
    bpeb = nc.dram_tensor("bpeb", [128, p.F], F32, kind="ExternalInput")
    discol = nc.dram_tensor("discol", [128, p.blocks], F32, kind="ExternalInput")
    ddiag = nc.dram_tensor("ddiag", [p.blocks * 128, p.bd], TDT, kind="ExternalInput")
    disinv = nc.dram_tensor("disinv", [1, p.blocks * p.bd], BF16, kind="ExternalInput")
    b1r = nc.dram_tensor("b1r", [1, p.F], BF16, kind="ExternalInput")
    iota = nc.dram_tensor("iota", [128, 128], BF16, kind="ExternalInput")
    ident = nc.dram_tensor("ident", [128, 128], BF16, kind="ExternalInput")
    vtag = None
    if p.vtag:
        vtag = nc.dram_tensor("vtag", [1, p.vtag], F32, kind="ExternalInput")
    idx = nc.dram_tensor("idx", [128, p.ncalls * (p.NI // 16)], I16, kind="ExternalInput")
    seg = nc.dram_tensor("seg", [128, p.ncalls * p.T], F32, kind="ExternalInput")
    out = nc.dram_tensor("out", [p.slots, p.F], F32, kind="ExternalOutput")

    with tile.TileContext(nc) as tc, ExitStack() as ctx:
        const = ctx.enter_context(tc.tile_pool(name="const", bufs=1))
        dram = ctx.enter_context(tc.tile_pool(name="dram", bufs=1, space="DRAM"))
        zpool = ctx.enter_context(tc.tile_pool(name="z", bufs=1))
        gpool = ctx.enter_context(tc.tile_pool(name="g", bufs=p.gbufs))
        spool = ctx.enter_context(tc.tile_pool(name="s", bufs=p.sbufs))
        hpool = ctx.enter_context(tc.tile_pool(name="h", bufs=p.hbufs))
        xpool = ctx.enter_context(tc.tile_pool(name="x", bufs=3))
        ppool = ctx.enter_context(tc.tile_pool(name="ps", bufs=max(3, p.mb + 1),
                                               space="PSUM"))
        p2pool = ctx.enter_context(tc.tile_pool(name="ps2", bufs=2, space="PSUM"))
        tppool = ctx.enter_context(tc.tile_pool(name="pst", bufs=1, space="PSUM"))

        w1_t = const.tile([128, p.kin * p.F], BF16)
        for k in range(p.kin):
            nc.sync.dma_start(w1_t[:, k * p.F:(k + 1) * p.F], W1[k * 128:(k + 1) * 128, :])
        w2_t = const.tile([128, p.kf * p.F], BF16)
        for k in range(p.kf):
            nc.sync.dma_start(w2_t[:, k * p.F:(k + 1) * p.F], W2[k * 128:(k + 1) * 128, :])
        wp_t = const.tile([128, p.kf * p.F], BF16)
        for k in range(p.kf):
            nc.sync.dma_start(wp_t[:, k * p.F:(k + 1) * p.F], Wp[k * 128:(k + 1) * 128, :])
        b1_t = const.tile([128, p.F], F32)
        nc.sync.dma_start(b1_t[:], b1b[:])
        bpe_t = const.tile([128, p.F], F32)
        nc.sync.dma_start(bpe_t[:], bpeb[:])
        dis_t = const.tile([128, p.blocks], F32)
        nc.sync.dma_start(dis_t[:], discol[:])
        ddiag_t = const.tile([128, p.blocks * p.bd], TDT)
        for b in range(p.blocks):
            nc.sync.dma_start(ddiag_t[:, b * p.bd:(b + 1) * p.bd],
                              ddiag[b * 128:(b + 1) * 128, :])
        disinv_t = const.tile([1, p.blocks * p.bd], BF16)
        nc.sync.dma_start(disinv_t[:], disinv[:])
        b1r_t = const.tile([1, p.F], BF16)
        nc.sync.dma_start(b1r_t[:], b1r[:])
        if vtag is not None:
            vtag_t = const.tile([1, p.vtag], F32)
            nc.sync.dma_start(vtag_t[:], vtag[:])
        iota_t = const.tile([128, 128], BF16)
        nc.sync.dma_start(iota_t[:], iota[:])
        ident_t = const.tile([128, 128], BF16)
        nc.sync.dma_start(ident_t[:], ident[:])
        idx_t = const.tile([128, p.ncalls * (p.NI // 16)], I16)
        nc.sync.dma_start(idx_t[:], idx[:])
        seg_t = const.tile([128, p.ncalls * p.T], F32)
        nc.sync.dma_start(seg_t[:], seg[:])

        z1loc = zpool.tile([128, p.blocks * p.F], TDT, tag="z1")
        z2loc = zpool.tile([128, p.blocks * p.F], TDT, tag="z2")

        zbounce1 = dram.tile([p.slots, p.F], TDT, tag="zb1")
        zbounce2 = dram.tile([p.slots, p.F], TDT, tag="zb2")
        ch = p.slots // 2
        bh = p.blocks // 2
        if p.ck:
            tab1a = nc.dram_tensor("tab1a", [p.half, p.F], TDT,
                                   kind="Internal", addr_space="Shared").ap()
            tab1b = nc.dram_tensor("tab1b", [p.half, p.F], TDT,
                                   kind="Internal", addr_space="Shared").ap()
            tab2a = nc.dram_tensor("tab2a", [p.half, p.F], TDT,
                                   kind="Internal", addr_space="Shared").ap()
            tab2b = nc.dram_tensor("tab2b", [p.half, p.F], TDT,
                                   kind="Internal", addr_space="Shared").ap()
            acc = zpool.tile([128, p.blocks * p.F], BF16, tag="acc")
        else:
            tab1 = nc.dram_tensor("tab1", [p.tbl_rows, p.F], TDT,
                                  kind="Internal", addr_space="Shared").ap()
            tab2 = nc.dram_tensor("tab2", [p.tbl_rows, p.F], TDT,
                                  kind="Internal", addr_space="Shared").ap()

        def allgather(src_ap, dst_ap):
            if p.c0:
                return
            nc.gpsimd.collective_compute(
                "AllGather", mybir.AluOpType.bypass,
                replica_groups=[list(range(p.n_cores))],
                ins=[src_ap], outs=[dst_ap])

        def produce_z1():
            for b in range(p.blocks):
                xt = xpool.tile([128, p.kin * p.bd], BF16, tag="xt")
                for k in range(p.kin):
                    nc.sync.dma_start(
                        xt[:, k * p.bd:(k + 1) * p.bd],
                        xT[k * 128:(k + 1) * 128, b * p.bd:(b + 1) * p.bd])
                ps = p2pool.tile([p.bd, p.F], F32, tag="zps")
                for k in range(p.kin):
                    nc.tensor.matmul(ps[:], xt[:, k * p.bd:(k + 1) * p.bd],
                                     w1_t[:, k * p.F:(k + 1) * p.F],
                                     start=(k == 0), stop=(k == p.kin - 1))
                zb = z1loc[:p.bd, b * p.F:(b + 1) * p.F]
                nc.vector.tensor_scalar_mul(zb, ps[:], dis_t[:p.bd, b:b + 1])
                nc.sync.dma_start(zbounce1[b * p.bd:(b + 1) * p.bd, :], zb)
                if p.ck and b == bh - 1:
                    allgather(zbounce1[0:ch, :].opt(), tab1a)
            if p.ck:
                allgather(zbounce1[ch:, :].opt(), tab1b)

        iota_b = iota_t[:].rearrange("p f -> p () f").broadcast_to([128, p.T, 128])

        gfix = sfix = None
        if p.g0:
            gfix = const.tile([128, p.mb * p.T, p.F], TDT)
            nc.vector.memset(gfix[:], 0.0)
        if p.s0:
            sfix = const.tile([128, p.T, 128], TDT)
            nc.vector.memset(sfix[:], 0.0)

        def agg_super(layer_tab, zloc, g, qc, l1=False):
            MB = p.mb
            gts = []
            for half in (0, 1):
                call = g * 2 + half
                if p.g0:
                    gts.append(gfix)
                    continue
                gt = gpool.tile([128, MB * p.T, p.F], TDT, tag="g")
                src = layer_tab[:p.half, :] if half == 0 else layer_tab[p.half:, :]
                w = MB * p.NI // 16
                nc.gpsimd.dma_gather(
                    gt[:], src, idx_t[:, call * w:(call + 1) * w],
                    MB * p.NI, MB * p.NI, p.F, queue_num=qc[0] % p.gq,
                    single_packet=p.single_packet)
                qc[0] += 1
                gts.append(gt)
            out_ps = []
            for bi in range(MB):
                b = g * MB + bi
                ps = ppool.tile([p.bd, p.F], F32, tag="agg")
                mm = 0
                for half, gt in ((0, gts[0]), (1, gts[1])):
                    sidx = (g * 2 + half) * MB + bi
                    if p.s0:
                        s = sfix
                    else:
                        s = spool.tile([128, p.T, 128], TDT, tag="s")
                        seg_b = seg_t[:, sidx * p.T:(sidx + 1) * p.T] \
                            .rearrange("p t -> p t ()").broadcast_to([128, p.T, 128])
                        nc.vector.tensor_tensor(s[:], iota_b, seg_b,
                                                op=mybir.AluOpType.is_equal)
                    if not p.m0:
                        for t in range(p.T):
                            nc.tensor.matmul(ps[:], s[:, t, :p.bd],
                                             gt[:, bi * p.T + t, :],
                                             start=(mm == 0), stop=False)
                            mm += 1
                if p.m0:
                    nc.tensor.matmul(ps[:], sfix[:, 0, :p.bd] if p.s0 else
                                     ddiag_t[:p.bd, b * p.bd:(b + 1) * p.bd],
                                     gfix[:, 0, :] if p.g0 else
                                     zloc[:p.bd, b * p.F:(b + 1) * p.F],
                                     start=True, stop=False)
                if l1:
                    nc.tensor.matmul(ps[:], disinv_t[:1, b * p.bd:(b + 1) * p.bd],
                                     b1r_t[:1, :], start=False, stop=False)
                nc.tensor.matmul(ps[:], ddiag_t[:p.bd, b * p.bd:(b + 1) * p.bd],
                                 zloc[:p.bd, b * p.F:(b + 1) * p.F],
                                 start=False, stop=True)
                out_ps.append((b, ps))
            return out_ps

        def gather_call(tab, call, qc):
            MB = p.mb
            gt = gpool.tile([128, MB * p.T, p.F], TDT, tag="g")
            w = MB * p.NI // 16
            nc.gpsimd.dma_gather(
                gt[:], tab, idx_t[:, call * w:(call + 1) * w],
                MB * p.NI, MB * p.NI, p.F, queue_num=qc[0] % p.gq,
                single_packet=p.single_packet)
            qc[0] += 1
            return gt

        def sbuild(sidx):
            s = spool.tile([128, p.T, 128], TDT, tag="s")
            seg_b = seg_t[:, sidx * p.T:(sidx + 1) * p.T] \
                .rearrange("p t -> p t ()").broadcast_to([128, p.T, 128])
            nc.vector.tensor_tensor(s[:], iota_b, seg_b,
                                    op=mybir.AluOpType.is_equal)
            return s

        def agg_phase_a(tabA, g, qc):
            gt = gather_call(tabA, g * 2, qc)
            for bi in range(p.mb):
                b = g * p.mb + bi
                ps = ppool.tile([p.bd, p.F], F32, tag="agg")
                s = sbuild((g * 2) * p.mb + bi)
                for t in range(p.T):
                    nc.tensor.matmul(ps[:], s[:, t, :p.bd],
                                     gt[:, bi * p.T + t, :],
                                     start=(t == 0), stop=(t == p.T - 1))
                nc.scalar.activation(acc[:p.bd, b * p.F:(b + 1) * p.F], ps[:],
                                     mybir.ActivationFunctionType.Copy)

        def agg_phase_b(tabB, zloc, g, qc, l1=False):
            gt = gather_call(tabB, g * 2 + 1, qc)
            out_ps = []
            for bi in range(p.mb):
                b = g * p.mb + bi
                ps = ppool.tile([p.bd, p.F], F32, tag="agg")
                s = sbuild((g * 2 + 1) * p.mb + bi)
                for t in range(p.T):
                    nc.tensor.matmul(ps[:], s[:, t, :p.bd],
                                     gt[:, bi * p.T + t, :],
                                     start=(t == 0), stop=False)
                if l1:
                    nc.tensor.matmul(ps[:], disinv_t[:1, b * p.bd:(b + 1) * p.bd],
                                     b1r_t[:1, :], start=False, stop=False)
                nc.tensor.matmul(ps[:], ddiag_t[:p.bd, b * p.bd:(b + 1) * p.bd],
                                 zloc[:p.bd, b * p.F:(b + 1) * p.F],
                                 start=False, stop=False)
                nc.tensor.matmul(ps[:], ident_t[:p.bd, :p.bd],
                                 acc[:p.bd, b * p.F:(b + 1) * p.F],
                                 start=False, stop=True)
                out_ps.append((b, ps))
            return out_ps

        def transpose_to(src_bf16):
            hT = hpool.tile([128, p.kf * 128], BF16, tag="ht")
            for k in range(p.kf):
                pst = tppool.tile([128, 128], BF16, tag="tps")
                nc.tensor.transpose(pst[:, :p.bd],
                                    src_bf16[:p.bd, k * 128:(k + 1) * 128],
                                    ident_t[:p.bd, :p.bd])
                nc.scalar.activation(hT[:, k * 128:k * 128 + p.bd], pst[:, :p.bd],
                                     mybir.ActivationFunctionType.Copy)
            return hT

        def layer1_block_post(b, ps):
            h1 = hpool.tile([128, p.F], BF16, tag="hb")
            nc.vector.tensor_scalar(h1[:p.bd], ps[:], dis_t[:p.bd, b:b + 1], 0.0,
                                    op0=mybir.AluOpType.mult,
                                    op1=mybir.AluOpType.max)
            h1T = transpose_to(h1)
            ps2 = p2pool.tile([p.bd, p.F], F32, tag="zps")
            for k in range(p.kf):
                nc.tensor.matmul(ps2[:], h1T[:, k * 128:k * 128 + p.bd],
                                 w2_t[:, k * p.F:(k + 1) * p.F],
                                 start=(k == 0), stop=(k == p.kf - 1))
            zb = z2loc[:p.bd, b * p.F:(b + 1) * p.F]
            nc.vector.tensor_scalar_mul(zb, ps2[:], dis_t[:p.bd, b:b + 1])
            nc.sync.dma_start(zbounce2[b * p.bd:(b + 1) * p.bd, :], zb)

        def layer2_block_post(b, ps):
            h2 = hpool.tile([128, p.F], BF16, tag="hb")
            nc.vector.tensor_scalar_mul(h2[:p.bd], ps[:], dis_t[:p.bd, b:b + 1])
            h2T = transpose_to(h2)
            ps2 = p2pool.tile([p.bd, p.F], F32, tag="zps")
            for k in range(p.kf):
                nc.tensor.matmul(ps2[:], h2T[:, k * 128:k * 128 + p.bd],
                                 wp_t[:, k * p.F:(k + 1) * p.F],
                                 start=(k == 0), stop=(k == p.kf - 1))
            of = hpool.tile([p.bd, p.F], F32, tag="of")
            nc.vector.tensor_add(of[:], ps2[:], bpe_t[:p.bd])
            nc.sync.dma_start(out[b * p.bd:(b + 1) * p.bd, :], of[:])

        qc = [0]
        for rep in range(p.reps):
            if p.ck:
                produce_z1()  # emits AG1a mid-loop + AG1b at end
                for g in range(p.blocks // p.mb):
                    agg_phase_a(tab1a, g, qc)
                for g in range(p.blocks // p.mb):
                    for b, ps in agg_phase_b(tab1b, z1loc, g, qc, l1=True):
                        layer1_block_post(b, ps)
                        if b == bh - 1:
                            allgather(zbounce2[0:ch, :].opt(), tab2a)
                allgather(zbounce2[ch:, :].opt(), tab2b)
                for g in range(p.blocks // p.mb):
                    agg_phase_a(tab2a, g, qc)
                for g in range(p.blocks // p.mb):
                    for b, ps in agg_phase_b(tab2b, z2loc, g, qc):
                        layer2_block_post(b, ps)
                continue

            produce_z1()
            if not p.c0:
                nc.gpsimd.collective_compute(
                    "AllGather", mybir.AluOpType.bypass,
                    replica_groups=[list(range(p.n_cores))],
                    ins=[zbounce1.opt()], outs=[tab1])

            for g in range(p.blocks // p.mb):
                for b, ps in agg_super(tab1, z1loc, g, qc, l1=True):
                    layer1_block_post(b, ps)

            if not p.c0:
                nc.gpsimd.collective_compute(
                    "AllGather", mybir.AluOpType.bypass,
                    replica_groups=[list(range(p.n_cores))],
                    ins=[zbounce2.opt()], outs=[tab2])

            for g in range(p.blocks // p.mb):
                for b, ps in agg_super(tab2, z2loc, g, qc):
                    layer2_block_post(b, ps)

    nc.compile()
    return nc


def _wrap_idx(flat, NI):
    a = np.empty((128, NI // 16), np.int16)
    blk = flat.reshape(NI // 16, 16).T
    for g in range(8):
        a[g * 16:(g + 1) * 16, :] = blk
    return a


def _preprocess(p, x, edge_index, W1, b1, W2, b2, Wp, bp):
    n = p.n_nodes
    src = np.asarray(edge_index[0], np.int64)
    dst = np.asarray(edge_index[1], np.int64)

    deg = np.bincount(dst, minlength=n).astype(np.float64) + 1.0
    dis = (1.0 / np.sqrt(deg)).astype(np.float32)

    src_t = _src_map(p, src)
    dst_core = dst // p.npc
    dst_slot = dst % p.npc
    dst_block = dst_slot // p.bd
    dst_seg = dst_slot % p.bd

    halfsel = (src_t >= p.half).astype(np.int64)
    bucket = dst_core * (p.blocks * 2) + dst_block * 2 + halfsel

    order = np.lexsort((src_t, bucket))
    b_sorted = bucket[order]
    s_sorted = src_t[order]
    g_sorted = dst_seg[order]

    nbuckets = p.n_cores * p.blocks * 2
    counts = np.bincount(b_sorted, minlength=nbuckets)
    assert int(counts.max()) <= p.NI
    starts = np.zeros(nbuckets + 1, np.int64)
    np.cumsum(counts, out=starts[1:])

    rtcnt = getattr(p, "rtcnt", 0)
    fill = -1 if rtcnt else 0
    idx_pad = np.full((nbuckets, p.NI), fill, np.int64)
    seg_pad = np.full((nbuckets, p.NI), -1.0, np.float32)
    cnt_arr = np.maximum(counts, 1).astype(np.int32)
    for bkt in range(nbuckets):
        s, e = starts[bkt], starts[bkt + 1]
        loc = s_sorted[s:e] - (p.half if bkt % 2 else 0)
        idx_pad[bkt, :e - s] = loc
        seg_pad[bkt, :e - s] = g_sorted[s:e]
        if rtcnt and e == s:
            idx_pad[bkt, 0] = 0

    W1b = np.asarray(W1, np.float32).astype(NPBF16)
    W2b = np.asarray(W2, np.float32).astype(NPBF16)
    Wpb = np.asarray(Wp, np.float32).astype(NPBF16)
    bpe = (np.asarray(b2, np.float32) @ np.asarray(Wp, np.float32)
           + np.asarray(bp, np.float32))
    b1b = np.broadcast_to(np.asarray(b1, np.float32), (128, p.F)).copy()
    bpeb = np.broadcast_to(bpe, (128, p.F)).copy()
    iota = np.broadcast_to(np.arange(128, dtype=np.float32), (128, 128)).astype(NPBF16)
    ident = np.eye(128, dtype=np.float32).astype(NPBF16)

    x = np.asarray(x, np.float32)
    in_maps = []
    for c in range(p.n_cores):
        lo, hi = c * p.npc, (c + 1) * p.npc
        xl = np.zeros((p.slots, p.in_dim), np.float32)
        xl[:p.npc] = x[lo:hi]
        xT = np.ascontiguousarray(xl.T).astype(NPBF16)

        disl = np.zeros(p.slots, np.float32)
        disl[:p.npc] = dis[lo:hi]
        dv = disl.reshape(p.blocks, p.bd)
        discol = np.zeros((128, p.blocks), np.float32)
        discol[:p.bd, :] = dv.T
        ddiag = np.zeros((p.blocks * 128, p.bd), np.float32)
        for b in range(p.blocks):
            np.fill_diagonal(ddiag[b * 128: b * 128 + p.bd, :],
                             (dv[b] > 0).astype(np.float32))
        ddiag = ddiag.astype(NPFP8 if p.fp8 else NPBF16)

        idx_sb = np.empty((128, p.ncalls * (p.NI // 16)), np.int16)
        seg_sb = np.empty((128, p.ncalls * p.T), np.float32)
        base = c * p.blocks * 2
        w = p.mb * p.NI // 16
        for g in range(p.blocks // p.mb):
            for half in (0, 1):
                gc = g * 2 + half
                flat = np.concatenate(
                    [idx_pad[base + (g * p.mb + bi) * 2 + half]
                     for bi in range(p.mb)])
                idx_sb[:, gc * w:(gc + 1) * w] = \
                    _wrap_idx(flat.astype(np.int16), p.mb * p.NI)
                for bi in range(p.mb):
                    sidx = gc * p.mb + bi
                    seg_sb[:, sidx * p.T:(sidx + 1) * p.T] = \
                        seg_pad[base + (g * p.mb + bi) * 2 + half] \
                        .reshape(p.T, 128).T
        disinv = np.zeros((1, p.slots), np.float32)
        nz = disl > 0
        disinv[0, nz] = 1.0 / disl[nz]
        extra = {}
        if p.vtag:
            extra["vtag"] = np.zeros((1, p.vtag), np.float32)
        in_maps.append({
            **extra,
            "xT": xT, "W1": W1b, "W2": W2b, "Wp": Wpb,
            "b1b": b1b, "bpeb": bpeb, "discol": discol, "ddiag": ddiag,
            "disinv": disinv.astype(NPBF16),
            "b1r": np.asarray(b1, np.float32).reshape(1, p.F).astype(NPBF16),
            "iota": iota, "ident": ident, "idx": idx_sb, "seg": seg_sb,
            "cnt": cnt_arr[base:base + p.ncalls][None, :],
        })
    return in_maps


_CACHE = {}

# chosen config: multi-packet SWDGE gathers (measurably faster than
# single-packet) + a small dummy input that keeps this program's HLO
# distinct from prior builds sharing the same tensor shapes.
FLAGS = dict(sp=False, vtag=24, block_dst=128)


def kernel(x, edge_index, W1, b1, W2, b2, Wp, bp):
    """Full inputs in, full output out. Shards across 8 NeuronCores inside."""
    from concourse.bass_utils import run_bass_kernel_spmd

    src = np.asarray(edge_index[0], np.int64)
    dst = np.asarray(edge_index[1], np.int64)

    # choose T (gather tiles per bucket) from the actual edge distribution;
    # identical across cores so the SPMD program stays uniform.
    p0 = _P(T=1, **FLAGS)
    src_t = _src_map(p0, src)
    halfsel = (src_t >= p0.half).astype(np.int64)
    dst_slot = dst % p0.npc
    bucket = ((dst // p0.npc) * (p0.blocks * 2)
              + (dst_slot // p0.bd) * 2 + halfsel)
    counts = np.bincount(bucket, minlength=p0.n_cores * p0.blocks * 2)
    T = max(1, -(-int(counts.max()) // 128))
    p = _P(T=T, **FLAGS)

    key = (T,)
    if key not in _CACHE:
        _CACHE[key] = _build_kernel(p)
    nc = _CACHE[key]

    in_maps = _preprocess(p, x, edge_index, W1, b1, W2, b2, Wp, bp)
    res = run_bass_kernel_spmd(nc, in_maps, core_ids=list(range(p.n_cores)))
    parts = [np.asarray(res.results[c]["out"])[:p.npc] for c in range(p.n_cores)]
    return np.concatenate(parts, axis=0).astype(np.float32)

